# revision 38
# baseline (speedup 1.0000x reference)
"""MoE BatchedExperts kernel for 8 trn2 NeuronCores.

Strategy: expert parallelism with host-side top-k dispatch. Each token has
exactly TOP_K nonzero routing weights, so core e only processes the tokens
routed to expert e (~N*K/E of them) instead of all N — 4x less compute than
the dense reference formulation, identical math (zero-score tokens
contribute zero).

Per core e (tokens gathered+retiled on host):
  hT = gelu(mm1 + b0)   [F, T]   mm1: lhsT=w0 tile [128,128], rhs=xT chunk
  y  = hT.T @ w1[e]     [T, D]   mm2: lhsT=hT tile [128,128], rhs=w1 chunk
Host combines: out[idx_e] += r_e * y_e rows; b1 folded in via routing @ b1.

All matmuls in fp16 (1 cycle/row like fp32r, ~1e-4 level rel err with fp32
PSUM accumulation, but: half the DMA bytes, and fp16 allows standalone
ldweights so mm1 can load each w0 k-tile once and stream all 3 t-chunks
through it — fp32r must re-load weights per matmul, which caps 256/384-col
matmuls at the ~187ns LDWEIGHTS rate instead of the stream rate).

All DMA'd tensors are pre-tiled on the host so every transfer is >=2KB
contiguous per partition (full ring rate; the old [D,T]-strided loads moved
0.5-1KB packets at half rate and cost 5us+ of descriptor generation).
"""

import numpy as np

import concourse.bacc as bacc
import concourse.mybir as mybir
from concourse.tile import TileContext
from concourse.bass_utils import run_bass_kernel_spmd

F32 = mybir.dt.float32
F16 = mybir.dt.float16

N, D, E, F = 4096, 1024, 8, 2048
P = 128
KD = D // P            # 8  k-tiles for mm1
KF = F // P            # 16 k-tiles for mm2
T_CHUNKS = [256, 384, 512]   # mm1 moving-dim chunks for T=1152
TCH = 384              # token pad granularity
D_CHUNKS = [512, 512]        # mm2 moving-dim chunks (sum = D)
KH = KF // 2                 # w1 streamed in (dc, k-half) tiles
RAMP_FOS = 5                 # fo's processed t-major while xT chunks stream in

# mm1 steady-state inner loop:
#  "ko":  k-outer/t-inner, self-loading matmuls (one w0 k-tile streams all
#         3 t-chunks; immune to slow-LDWEIGHTS outliers)
#  "kin": t-outer/k-inner (baseline order, control)
MM1_MODE = "ko"

_cache: dict[tuple, object] = {}


def build_program(T: int, mode: str):
    """Bass program for one expert shard with T_mm1 = `T` mm1 columns.

    mm1 computes only T columns (= max routed count, rounded to 16); the
    mm2 token tiles span TO*128 >= T, reading never-written h columns for
    the pad region — those only produce y rows the host discards.
    """
    TO = -(-T // P)
    t_chunks = _chunks_for(T)
    NTC = len(t_chunks)
    t_offs = [0, *np.cumsum(t_chunks).tolist()]

    nc = bacc.Bacc("TRN2", target_bir_lowering=False, debug=False)
    # all pre-tiled on host: per-partition-contiguous blocks
    xt = nc.dram_tensor("xt", [P, KD * T], F16, kind="ExternalInput")
    w0t = nc.dram_tensor("w0t", [P, KF * KD * P], F16, kind="ExternalInput")
    w1t = nc.dram_tensor("w1t", [P, 4 * KH * 512], F16, kind="ExternalInput")
    b0 = nc.dram_tensor("b0", [P, KF], F32, kind="ExternalInput")
    y = nc.dram_tensor("y", [TO * P, D], F16, kind="ExternalOutput")

    with TileContext(nc) as tc:
        with tc.tile_pool(name="const", bufs=1) as const, \
             tc.tile_pool(name="xpool", bufs=1) as xpool, \
             tc.tile_pool(name="hpool", bufs=1) as hpool, \
             tc.tile_pool(name="w0pool", bufs=1) as w0pool, \
             tc.tile_pool(name="w1pool", bufs=1) as w1pool, \
             tc.tile_pool(name="ypool", bufs=3) as ypool, \
             tc.tile_pool(name="psum", bufs=8, space="PSUM") as psum:

            # NOTE: gpsimd is deliberately untouched — any gpsimd activity
            # (memset/DMA) downclocks the tensor engine ~2.4 -> 2.0GHz.
            # scalar queue: x chunks + b0 (they gate the PE ramp; the 8MB
            # weight stream must not sit in front of them)
            warm = const.tile([P, P], F16, name="warm")
            nc.vector.memset(warm[:], 0.0)

            # DMA plan: both FIFO queues carry ramp-critical transfers in
            # consumption order. Early w0 tiles alternate between the two
            # queues (one queue alone delivers a 0.25MB tile each ~2us but
            # the t0 phase eats one per ~0.9us); everything not needed
            # until later (w0_5.., w1) is gated behind x2's arrival so the
            # x stream gets the full ring bandwidth when it matters.
            # DMA pacing: all outstanding transfers round-robin on the
            # rings, so issuing everything upfront starves the
            # ramp-critical x0/w0 head (~8 transfers in flight each get a
            # fraction of the bandwidth). Instead, transfers are split
            # into ~0.25MB pieces: each DMA_DIRECT2D issue costs ~0.6us of
            # engine time — about what a piece takes to transfer — so the
            # in-flight set stays small and in consumption order, and
            # subtile deps let matmuls start on partially-landed chunks.
            x_sb = []
            xtile = xpool.tile([P, KD * t_chunks[0]], F16, tag="x0", name="x0")
            nc.scalar.dma_start(xtile[:], xt[:, 0:KD * t_offs[1]])
            x_sb.append(xtile)
            b0_sb = const.tile([P, KF], F32)
            nc.scalar.dma_start(b0_sb[:], b0[:, :])
            for t in range(1, NTC):
                xtile = xpool.tile([P, KD * t_chunks[t]], F16, tag=f"x{t}",
                                   name=f"x{t}")
                nc.scalar.dma_start(
                    xtile[:], xt[:, KD * t_offs[t]:KD * t_offs[t + 1]])
                x_sb.append(xtile)

            # w0 as 16 per-tile transfers (a tile becomes usable the moment
            # it lands — grouped transfers stall the PE on whole groups);
            # w1 as one transfer, only needed at phase 2.
            w0_sb = []
            for fo in range(KF):
                wtile = w0pool.tile([P, KD * P], F16, tag=f"w0_{fo}",
                                    name=f"w0_{fo}")
                o = fo * KD * P
                nc.sync.dma_start(wtile[:], w0t[:, o:o + KD * P])
                w0_sb.append(wtile)
            w1_all = w1pool.tile([P, 4 * KH * 512], F16, tag="w1", name="w1")
            nc.sync.dma_start(w1_all[:], w1t[:, :])

            # PE p-state warmup: the tensor engine idles at ~1.2GHz and
            # ramps to 2.4GHz over ~3us of continuous activity. Junk
            # matmuls during the x0/w0_0 DMA wait start the ramp early so
            # the real ramp runs at full clock.
            wps = psum.tile([P, P], F32, tag="ps", name="warm_ps")
            for r in range(68):
                nc.tensor.matmul(wps[:, :64], warm[:], warm[:, :64],
                                 start=(r == 0), stop=(r == 67))
            wjunk = const.tile([P, 32], F32, name="warm_out")
            nc.vector.tensor_copy(wjunk[:], wps[:, :32])

            # hT = gelu(x @ w0 + b0), laid out [F-part, T-free], fp16
            h_sb = hpool.tile([P, KF, TO * P], F16)

            def x_k(t, k):
                return x_sb[t][:, k * t_chunks[t]:(k + 1) * t_chunks[t]]

            def w0_k(fo, k):
                return w0_sb[fo][:, k * P:(k + 1) * P]

            def act(fo, t, ps):
                nc.scalar.activation(h_sb[:, fo, t_offs[t]:t_offs[t + 1]], ps,
                                     mybir.ActivationFunctionType.Gelu,
                                     bias=b0_sb[:, fo:fo + 1])

            # ---- phase 1: mm1 + gelu ----
            # ramp: ordered to match measured DMA arrivals — x0, w0_0..2
            # land first, then x1, then w0_3/w0_4 (delayed while x1/x2 hog
            # rings), then x2. Each group is k-inner self-loading.
            if NTC >= 3:
                ramp_pairs = [(f, 0) for f in range(3)]
                ramp_pairs += [(f, 1) for f in range(3)]
                ramp_pairs += [(3, 0), (3, 1), (4, 0), (4, 1)]
                ramp_pairs += [(f, t) for t in range(2, NTC)
                               for f in range(RAMP_FOS)]
            else:
                ramp_pairs = [(f, t) for t in range(NTC)
                              for f in range(RAMP_FOS)]
            for fo, t in ramp_pairs:
                ps = psum.tile([P, 512], F32, tag="ps",
                               name=f"ps1_{fo}_{t}")[:, :t_chunks[t]]
                for k in range(KD):
                    nc.tensor.matmul(ps, w0_k(fo, k), x_k(t, k),
                                     start=(k == 0), stop=(k == KD - 1))
                act(fo, t, ps)

            # steady: k-outer/t-inner — each w0 k-tile streams all chunks
            for fo in range(RAMP_FOS, KF):
                if mode == "kin":
                    for t in range(NTC):
                        ps = psum.tile([P, 512], F32, tag="ps",
                                       name=f"ps1_{fo}_{t}")[:, :t_chunks[t]]
                        for k in range(KD):
                            nc.tensor.matmul(ps, w0_k(fo, k), x_k(t, k),
                                             start=(k == 0),
                                             stop=(k == KD - 1))
                        act(fo, t, ps)
                else:
                    pss = [psum.tile([P, 512], F32, tag="ps",
                                     name=f"ps1_{fo}_{t}")[:, :t_chunks[t]]
                           for t in range(NTC)]
                    for k in range(KD):
                        for t in range(NTC):
                            nc.tensor.matmul(pss[t], w0_k(fo, k), x_k(t, k),
                                             start=(k == 0),
                                             stop=(k == KD - 1))
                    for t in range(NTC):
                        act(fo, t, pss[t])

            # ---- phase 2: mm2 ----
            # the very last output group is split into two 256-wide halves
            # so the end-of-kernel copy+DMA tail is half as long
            for dc, DCH in enumerate(D_CHUNKS):
                for to in range(TO):
                    last = (dc == len(D_CHUNKS) - 1) and (to == TO - 1)
                    for half in ([None] if not last else [0, 1]):
                        if half is None:
                            d0, dw = 0, 512
                        else:
                            d0, dw = half * 256, 256
                        ps2 = psum.tile([P, 512], F32, tag="ps",
                                        name=f"ps2_{dc}_{to}_{d0}")[:, :dw]
                        for k in range(KF):
                            o = ((dc * 2 + k // KH) * KH
                                 + k % KH) * 512 + d0
                            nc.tensor.matmul(
                                ps2, h_sb[:, k, to * P:(to + 1) * P],
                                w1_all[:, o:o + dw],
                                start=(k == 0), stop=(k == KF - 1))
                        y_sb = ypool.tile([P, 512], F16, tag="y",
                                          name=f"y_{dc}_{to}_{d0}")[:, :dw]
                        nc.vector.tensor_copy(y_sb, ps2)
                        nc.sync.dma_start(
                            y[to * P:(to + 1) * P,
                              dc * 512 + d0:dc * 512 + d0 + dw], y_sb)

    nc.compile()
    return nc


def _chunks_for(T):
    """mm1 moving-dim chunks: small leading chunks for the DMA/PE ramp,
    512 (the PSUM bank limit) thereafter."""
    t_chunks = []
    rest = T
    for c in (192, 448):
        if rest <= 0:
            break
        t_chunks.append(min(c, rest))
        rest -= t_chunks[-1]
    while rest > 0:
        c = min(512, rest)
        t_chunks.append(c)
        rest -= c
    return t_chunks


def _prep_inputs(x, w0, b0, w1, idx, cnt, T, t_chunks, t_offs):
    """Host-side gather + fp16 cast + retile into per-partition-contiguous
    DMA blocks for one expert."""
    # x chunks: [P, KD*tc] blocks, chunk-major
    xT = np.zeros((KD, P, T), dtype=np.float16)
    xT.reshape(D, T)[:, :cnt] = x[idx].T
    parts = [xT[:, :, t_offs[c]:t_offs[c + 1]].transpose(1, 0, 2)
             .reshape(P, -1) for c in range(len(t_chunks))]
    xt = np.ascontiguousarray(np.concatenate(parts, axis=1))
    # w0 tiles: per fo [P, KD*128]
    w0t = np.ascontiguousarray(
        w0.astype(np.float16).reshape(KD, P, KF, P)
        .transpose(1, 2, 0, 3).reshape(P, KF * KD * P))
    # w1 tiles: per (dc, kh) [P, KH*512]
    w1t = np.ascontiguousarray(
        w1.astype(np.float16).reshape(2, KH, P, 2, 512)
        .transpose(2, 3, 0, 1, 4).reshape(P, 4 * KH * 512))
    b0t = np.ascontiguousarray(b0[0].astype(np.float32).reshape(KF, P).T)
    return {"xt": xt, "w0t": w0t, "w1t": w1t, "b0": b0t}


def kernel(x, routing_tensor, w0, b0, w1, b1):
    x = np.ascontiguousarray(np.asarray(x, dtype=np.float32))
    routing = np.asarray(routing_tensor, dtype=np.float32)
    w0 = np.asarray(w0, dtype=np.float32)
    b0 = np.asarray(b0, dtype=np.float32)
    w1 = np.asarray(w1, dtype=np.float32)
    b1 = np.asarray(b1, dtype=np.float32)

    idx = [np.nonzero(routing[:, e])[0] for e in range(E)]
    cnt = [len(i) for i in idx]
    T = max(256, -(-max(cnt) // 16) * 16)   # mm1 columns (pad to 16)
    t_chunks = _chunks_for(T)
    t_offs = [0, *np.cumsum(t_chunks).tolist()]

    nc = _cache.get((T, MM1_MODE))
    if nc is None:
        nc = _cache[T, MM1_MODE] = build_program(T, MM1_MODE)

    in_maps = [_prep_inputs(x, w0[e], b0[e], w1[e], idx[e], cnt[e], T,
                            t_chunks, t_offs) for e in range(E)]

    res = run_bass_kernel_spmd(nc, in_maps, core_ids=list(range(E)))

    # combine: out = sum_e r_e * (y_e + b1_e)
    out = routing @ b1[:, 0, :]
    for e in range(E):
        r = routing[idx[e], e:e + 1]
        out[idx[e]] += r * res.results[e]["y"][:cnt[e]].astype(np.float32)
    return out.astype(np.float32)


# revision 42
# speedup vs baseline: 1.1636x; 1.1636x over previous
"""MoE BatchedExperts kernel for 8 trn2 NeuronCores.

Strategy: expert parallelism with host-side top-k dispatch. Each token has
exactly TOP_K nonzero routing weights, so core e only processes the tokens
routed to expert e (~N*K/E of them) instead of all N — 4x less compute than
the dense reference formulation, identical math (zero-score tokens
contribute zero).

Per core e (tokens gathered+retiled on host):
  hT = gelu(mm1 + b0)   [F, T]   mm1: lhsT=w0 tile [128,128], rhs=xT chunk
  y  = hT.T @ w1[e]     [T, D]   mm2: lhsT=hT tile [128,128], rhs=w1 chunk
Host combines: out[idx_e] += r_e * y_e rows; b1 folded in via routing @ b1.

All matmuls in fp16: 1 cycle/row like fp32r and ~5e-4 rel err with fp32
PSUM accumulation, but half the DMA bytes and, critically, ~100ns
LDWEIGHTS (vs 187ns for fp32r's 4-byte loads) — so even 256-col matmuls
run at stream rate instead of being weight-load-bound.

All DMA'd tensors are pre-tiled on the host so every transfer is >=2KB
contiguous per partition (full ring rate; the old [D,T]-strided loads moved
0.5-1KB packets at half rate and cost 5us+ of descriptor generation).

Hardware behaviors this schedule is tuned around (measured via NTFF):
 - outstanding DMA transfers round-robin on the rings (~358GB/s total per
   core), so issue order/count controls what arrives first;
 - the tensor engine idles at ~1.2GHz and needs ~3us of activity to reach
   2.4GHz (hence the junk-matmul warmup during the first DMA wait);
 - any gpsimd activity (memset/DMA) downclocks the tensor engine to
   ~2.0GHz for the whole kernel — gpsimd is deliberately untouched.
"""

import numpy as np

import concourse.bacc as bacc
import concourse.mybir as mybir
from concourse.tile import TileContext
from concourse.bass_utils import run_bass_kernel_spmd

F32 = mybir.dt.float32
F16 = mybir.dt.float16

N, D, E, F = 4096, 1024, 8, 2048
P = 128
KD = D // P            # 8  k-tiles for mm1
KF = F // P            # 16 k-tiles for mm2
T_CHUNKS = [256, 384, 512]   # mm1 moving-dim chunks for T=1152
TCH = 384              # token pad granularity
D_CHUNKS = [512, 512]        # mm2 moving-dim chunks (sum = D)
KH = KF // 2                 # w1 streamed in (dc, k-half) tiles
RAMP_FOS = 5                 # fo's processed t-major while xT chunks stream in

# mm1 steady-state inner loop:
#  "ko":  k-outer/t-inner, self-loading matmuls (one w0 k-tile streams all
#         3 t-chunks; immune to slow-LDWEIGHTS outliers)
#  "kin": t-outer/k-inner (baseline order, control)
MM1_MODE = "ko"

_cache: dict[tuple, object] = {}


def build_program(T: int, mode: str):
    """Bass program for one expert shard with T_mm1 = `T` mm1 columns.

    mm1 computes only T columns (= max routed count, rounded to 16); the
    mm2 token tiles span TO*128 >= T, reading never-written h columns for
    the pad region — those only produce y rows the host discards.
    """
    TO = -(-T // P)
    t_chunks = _chunks_for(T)
    NTC = len(t_chunks)
    t_offs = [0, *np.cumsum(t_chunks).tolist()]

    nc = bacc.Bacc("TRN2", target_bir_lowering=False, debug=False)
    # all pre-tiled on host: per-partition-contiguous blocks
    xt = nc.dram_tensor("xt", [P, KD * T], F16, kind="ExternalInput")
    w0t = nc.dram_tensor("w0t", [P, KF * KD * P], F16, kind="ExternalInput")
    w1t = nc.dram_tensor("w1t", [P, 4 * KH * 512], F16, kind="ExternalInput")
    b0 = nc.dram_tensor("b0", [P, KF], F32, kind="ExternalInput")
    y = nc.dram_tensor("y", [TO * P, D], F16, kind="ExternalOutput")

    with TileContext(nc) as tc:
        with tc.tile_pool(name="const", bufs=1) as const, \
             tc.tile_pool(name="xpool", bufs=1) as xpool, \
             tc.tile_pool(name="hpool", bufs=1) as hpool, \
             tc.tile_pool(name="w0pool", bufs=1) as w0pool, \
             tc.tile_pool(name="w1pool", bufs=1) as w1pool, \
             tc.tile_pool(name="ypool", bufs=3) as ypool, \
             tc.tile_pool(name="psum", bufs=8, space="PSUM") as psum:

            warm = const.tile([P, P], F16, name="warm")
            nc.vector.memset(warm[:], 0.0)

            # scalar queue: x chunks + b0 (they gate the PE ramp; the 8MB
            # weight stream must not sit in front of them on the rings)
            x_sb = []
            xtile = xpool.tile([P, KD * t_chunks[0]], F16, tag="x0", name="x0")
            nc.scalar.dma_start(xtile[:], xt[:, 0:KD * t_offs[1]])
            x_sb.append(xtile)
            b0_sb = const.tile([P, KF], F32)
            nc.scalar.dma_start(b0_sb[:], b0[:, :])
            for t in range(1, NTC):
                xtile = xpool.tile([P, KD * t_chunks[t]], F16, tag=f"x{t}",
                                   name=f"x{t}")
                nc.scalar.dma_start(
                    xtile[:], xt[:, KD * t_offs[t]:KD * t_offs[t + 1]])
                x_sb.append(xtile)

            # w0 as 16 per-tile transfers (a tile becomes usable the moment
            # it lands — grouped transfers stall the PE on whole groups);
            # w1 as one transfer, only needed at phase 2.
            w0_sb = []
            for fo in range(KF):
                wtile = w0pool.tile([P, KD * P], F16, tag=f"w0_{fo}",
                                    name=f"w0_{fo}")
                o = fo * KD * P
                nc.sync.dma_start(wtile[:], w0t[:, o:o + KD * P])
                w0_sb.append(wtile)
            w1_all = w1pool.tile([P, 4 * KH * 512], F16, tag="w1", name="w1")
            nc.sync.dma_start(w1_all[:], w1t[:, :])

            # PE p-state warmup: the tensor engine idles at ~1.2GHz and
            # ramps to 2.4GHz over ~3us of continuous activity. Junk
            # matmuls during the x0/w0_0 DMA wait start the ramp early so
            # the real ramp runs at full clock.
            wps = psum.tile([P, P], F32, tag="ps", name="warm_ps")
            for r in range(48):
                nc.tensor.matmul(wps[:, :64], warm[:], warm[:, :64],
                                 start=(r == 0), stop=(r == 47))
            wjunk = const.tile([P, 32], F32, name="warm_out")
            nc.vector.tensor_copy(wjunk[:], wps[:, :32])

            # hT = gelu(x @ w0 + b0), laid out [F-part, T-free], fp16
            h_sb = hpool.tile([P, KF, TO * P], F16)

            def x_k(t, k):
                return x_sb[t][:, k * t_chunks[t]:(k + 1) * t_chunks[t]]

            def w0_k(fo, k):
                return w0_sb[fo][:, k * P:(k + 1) * P]

            def act(fo, t, ps):
                nc.scalar.activation(h_sb[:, fo, t_offs[t]:t_offs[t + 1]], ps,
                                     mybir.ActivationFunctionType.Gelu,
                                     bias=b0_sb[:, fo:fo + 1])

            # ---- phase 1: mm1 + gelu ----
            # ramp: ordered to match measured DMA arrivals — x0, w0_0..2
            # land first, then x1, then w0_3/w0_4 (delayed while x1/x2 hog
            # rings), then x2. Each group is k-inner self-loading.
            if NTC >= 3:
                ramp_pairs = [(f, 0) for f in range(3)]
                ramp_pairs += [(f, 1) for f in range(3)]
                ramp_pairs += [(3, 0), (3, 1), (4, 0), (4, 1)]
                ramp_pairs += [(f, t) for t in range(2, NTC)
                               for f in range(RAMP_FOS)]
            else:
                ramp_pairs = [(f, t) for t in range(NTC)
                              for f in range(RAMP_FOS)]
            for fo, t in ramp_pairs:
                ps = psum.tile([P, 512], F32, tag="ps",
                               name=f"ps1_{fo}_{t}")[:, :t_chunks[t]]
                for k in range(KD):
                    nc.tensor.matmul(ps, w0_k(fo, k), x_k(t, k),
                                     start=(k == 0), stop=(k == KD - 1))
                act(fo, t, ps)

            # steady: k-outer/t-inner — each w0 k-tile streams all chunks
            for fo in range(RAMP_FOS, KF):
                if mode == "kin":
                    for t in range(NTC):
                        ps = psum.tile([P, 512], F32, tag="ps",
                                       name=f"ps1_{fo}_{t}")[:, :t_chunks[t]]
                        for k in range(KD):
                            nc.tensor.matmul(ps, w0_k(fo, k), x_k(t, k),
                                             start=(k == 0),
                                             stop=(k == KD - 1))
                        act(fo, t, ps)
                else:
                    pss = [psum.tile([P, 512], F32, tag="ps",
                                     name=f"ps1_{fo}_{t}")[:, :t_chunks[t]]
                           for t in range(NTC)]
                    for k in range(KD):
                        for t in range(NTC):
                            nc.tensor.matmul(pss[t], w0_k(fo, k), x_k(t, k),
                                             start=(k == 0),
                                             stop=(k == KD - 1))
                    for t in range(NTC):
                        act(fo, t, pss[t])

            # ---- phase 2: mm2 ----
            # the very last output group is split into two 256-wide halves
            # so the end-of-kernel copy+DMA tail is half as long
            for dc, DCH in enumerate(D_CHUNKS):
                for to in range(TO):
                    last = (dc == len(D_CHUNKS) - 1) and (to == TO - 1)
                    for half in ([None] if not last else [0, 1]):
                        if half is None:
                            d0, dw = 0, 512
                        else:
                            d0, dw = half * 256, 256
                        ps2 = psum.tile([P, 512], F32, tag="ps",
                                        name=f"ps2_{dc}_{to}_{d0}")[:, :dw]
                        for k in range(KF):
                            o = ((dc * 2 + k // KH) * KH
                                 + k % KH) * 512 + d0
                            nc.tensor.matmul(
                                ps2, h_sb[:, k, to * P:(to + 1) * P],
                                w1_all[:, o:o + dw],
                                start=(k == 0), stop=(k == KF - 1))
                        y_sb = ypool.tile([P, 512], F16, tag="y",
                                          name=f"y_{dc}_{to}_{d0}")[:, :dw]
                        nc.vector.tensor_copy(y_sb, ps2)
                        nc.sync.dma_start(
                            y[to * P:(to + 1) * P,
                              dc * 512 + d0:dc * 512 + d0 + dw], y_sb)

    nc.compile()
    return nc


def _chunks_for(T):
    """mm1 moving-dim chunks: small leading chunks for the DMA/PE ramp,
    512 (the PSUM bank limit) thereafter."""
    t_chunks = []
    rest = T
    for c in (256, 384):
        if rest <= 0:
            break
        t_chunks.append(min(c, rest))
        rest -= t_chunks[-1]
    while rest > 0:
        c = min(512, rest)
        t_chunks.append(c)
        rest -= c
    return t_chunks


def _prep_inputs(x, w0, b0, w1, idx, cnt, T, t_chunks, t_offs):
    """Host-side gather + fp16 cast + retile into per-partition-contiguous
    DMA blocks for one expert."""
    # x chunks: [P, KD*tc] blocks, chunk-major
    xT = np.zeros((KD, P, T), dtype=np.float16)
    xT.reshape(D, T)[:, :cnt] = x[idx].T
    parts = [xT[:, :, t_offs[c]:t_offs[c + 1]].transpose(1, 0, 2)
             .reshape(P, -1) for c in range(len(t_chunks))]
    xt = np.ascontiguousarray(np.concatenate(parts, axis=1))
    # w0 tiles: per fo [P, KD*128]
    w0t = np.ascontiguousarray(
        w0.astype(np.float16).reshape(KD, P, KF, P)
        .transpose(1, 2, 0, 3).reshape(P, KF * KD * P))
    # w1 tiles: per (dc, kh) [P, KH*512]
    w1t = np.ascontiguousarray(
        w1.astype(np.float16).reshape(2, KH, P, 2, 512)
        .transpose(2, 3, 0, 1, 4).reshape(P, 4 * KH * 512))
    b0t = np.ascontiguousarray(b0[0].astype(np.float32).reshape(KF, P).T)
    return {"xt": xt, "w0t": w0t, "w1t": w1t, "b0": b0t}


def kernel(x, routing_tensor, w0, b0, w1, b1):
    x = np.ascontiguousarray(np.asarray(x, dtype=np.float32))
    routing = np.asarray(routing_tensor, dtype=np.float32)
    w0 = np.asarray(w0, dtype=np.float32)
    b0 = np.asarray(b0, dtype=np.float32)
    w1 = np.asarray(w1, dtype=np.float32)
    b1 = np.asarray(b1, dtype=np.float32)

    idx = [np.nonzero(routing[:, e])[0] for e in range(E)]
    cnt = [len(i) for i in idx]
    T = max(256, -(-max(cnt) // 16) * 16)   # mm1 columns (pad to 16)
    t_chunks = _chunks_for(T)
    t_offs = [0, *np.cumsum(t_chunks).tolist()]

    nc = _cache.get((T, MM1_MODE))
    if nc is None:
        nc = _cache[T, MM1_MODE] = build_program(T, MM1_MODE)

    in_maps = [_prep_inputs(x, w0[e], b0[e], w1[e], idx[e], cnt[e], T,
                            t_chunks, t_offs) for e in range(E)]

    res = run_bass_kernel_spmd(nc, in_maps, core_ids=list(range(E)))

    # combine: out = sum_e r_e * (y_e + b1_e)
    out = routing @ b1[:, 0, :]
    for e in range(E):
        r = routing[idx[e], e:e + 1]
        out[idx[e]] += r * res.results[e]["y"][:cnt[e]].astype(np.float32)
    return out.astype(np.float32)


# revision 43
# speedup vs baseline: 1.1690x; 1.0046x over previous
"""MoE BatchedExperts kernel for 8 trn2 NeuronCores.

Strategy: expert parallelism with host-side top-k dispatch. Each token has
exactly TOP_K nonzero routing weights, so core e only processes the tokens
routed to expert e (~N*K/E of them) instead of all N — 4x less compute than
the dense reference formulation, identical math (zero-score tokens
contribute zero).

Per core e (tokens gathered+retiled on host):
  hT = gelu(mm1 + b0)   [F, T]   mm1: lhsT=w0 tile [128,128], rhs=xT chunk
  y  = hT.T @ w1[e]     [T, D]   mm2: lhsT=hT tile [128,128], rhs=w1 chunk
Host combines: out[idx_e] += r_e * y_e rows; b1 folded in via routing @ b1.

All matmuls in fp16: 1 cycle/row like fp32r and ~5e-4 rel err with fp32
PSUM accumulation, but half the DMA bytes and, critically, ~100ns
LDWEIGHTS (vs 187ns for fp32r's 4-byte loads) — so even 256-col matmuls
run at stream rate instead of being weight-load-bound.

All DMA'd tensors are pre-tiled on the host so every transfer is >=2KB
contiguous per partition (full ring rate; the old [D,T]-strided loads moved
0.5-1KB packets at half rate and cost 5us+ of descriptor generation).

Hardware behaviors this schedule is tuned around (measured via NTFF):
 - outstanding DMA transfers round-robin on the rings (~358GB/s total per
   core), so issue order/count controls what arrives first;
 - the tensor engine idles at ~1.2GHz and needs ~3us of activity to reach
   2.4GHz (hence the junk-matmul warmup during the first DMA wait);
 - any gpsimd activity (memset/DMA) downclocks the tensor engine to
   ~2.0GHz for the whole kernel — gpsimd is deliberately untouched.
"""

import numpy as np

import concourse.bacc as bacc
import concourse.mybir as mybir
from concourse.tile import TileContext
from concourse.bass_utils import run_bass_kernel_spmd

F32 = mybir.dt.float32
F16 = mybir.dt.float16

N, D, E, F = 4096, 1024, 8, 2048
P = 128
KD = D // P            # 8  k-tiles for mm1
KF = F // P            # 16 k-tiles for mm2
T_CHUNKS = [256, 384, 512]   # mm1 moving-dim chunks for T=1152
TCH = 384              # token pad granularity
D_CHUNKS = [512, 512]        # mm2 moving-dim chunks (sum = D)
KH = KF // 2                 # w1 streamed in (dc, k-half) tiles
RAMP_FOS = 5                 # fo's processed t-major while xT chunks stream in

# mm1 steady-state inner loop:
#  "ko":  k-outer/t-inner, self-loading matmuls (one w0 k-tile streams all
#         3 t-chunks; immune to slow-LDWEIGHTS outliers)
#  "kin": t-outer/k-inner (baseline order, control)
MM1_MODE = "ko"

_cache: dict[tuple, object] = {}


def build_program(T: int, mode: str):
    """Bass program for one expert shard with T_mm1 = `T` mm1 columns.

    mm1 computes only T columns (= max routed count, rounded to 16); the
    mm2 token tiles span TO*128 >= T, reading never-written h columns for
    the pad region — those only produce y rows the host discards.
    """
    TO = -(-T // P)
    t_chunks = _chunks_for(T)
    NTC = len(t_chunks)
    t_offs = [0, *np.cumsum(t_chunks).tolist()]

    nc = bacc.Bacc("TRN2", target_bir_lowering=False, debug=False)
    # all pre-tiled on host: per-partition-contiguous blocks
    xt = nc.dram_tensor("xt", [P, KD * T], F16, kind="ExternalInput")
    w0t = nc.dram_tensor("w0t", [P, KF * KD * P], F16, kind="ExternalInput")
    w1t = nc.dram_tensor("w1t", [P, 4 * KH * 512], F16, kind="ExternalInput")
    b0 = nc.dram_tensor("b0", [P, KF], F32, kind="ExternalInput")
    y = nc.dram_tensor("y", [TO * P, D], F16, kind="ExternalOutput")

    with TileContext(nc) as tc:
        with tc.tile_pool(name="const", bufs=1) as const, \
             tc.tile_pool(name="xpool", bufs=1) as xpool, \
             tc.tile_pool(name="hpool", bufs=1) as hpool, \
             tc.tile_pool(name="w0pool", bufs=1) as w0pool, \
             tc.tile_pool(name="w1pool", bufs=1) as w1pool, \
             tc.tile_pool(name="ypool", bufs=3) as ypool, \
             tc.tile_pool(name="psum", bufs=8, space="PSUM") as psum:

            warm = const.tile([P, P], F16, name="warm")
            nc.vector.memset(warm[:], 0.0)

            # Outstanding transfers round-robin on the rings, so issuing
            # everything upfront gives the ramp-critical head (x0, w0_0/1)
            # only a fraction of the bandwidth. Pacing: a dummy SBUF->SBUF
            # dma issue burns ~0.6us of real engine time and carries no
            # semaphore wait (warm is ready before any issue), so the
            # scheduler cannot hoist later DMAs past it — later transfers
            # are issued roughly when the stream is ready for them.
            _pace_n = [0]

            def pace(engine, n):
                for _ in range(n):
                    i = _pace_n[0] = _pace_n[0] + 1
                    dt_ = const.tile([P, 1], F16, name=f"pace{i}")
                    engine.dma_start(dt_[:], warm[:, 0:1])

            # scalar queue: x chunks + b0 (they gate the PE ramp; the 8MB
            # weight stream must not sit in front of them on the rings)
            x_sb = []
            xtile = xpool.tile([P, KD * t_chunks[0]], F16, tag="x0", name="x0")
            nc.scalar.dma_start(xtile[:], xt[:, 0:KD * t_offs[1]])
            x_sb.append(xtile)
            pace(nc.scalar, 3)
            b0_sb = const.tile([P, KF], F32)
            for t in range(1, NTC):
                xtile = xpool.tile([P, KD * t_chunks[t]], F16, tag=f"x{t}",
                                   name=f"x{t}")
                nc.scalar.dma_start(
                    xtile[:], xt[:, KD * t_offs[t]:KD * t_offs[t + 1]])
                x_sb.append(xtile)
                if t == 1:
                    nc.scalar.dma_start(b0_sb[:], b0[:, :])
                    pace(nc.scalar, 2)

            # w0 as 16 per-tile transfers (a tile becomes usable the moment
            # it lands — grouped transfers stall the PE on whole groups);
            # w1 as one transfer, only needed at phase 2.
            w0_sb = []
            for fo in range(KF):
                wtile = w0pool.tile([P, KD * P], F16, tag=f"w0_{fo}",
                                    name=f"w0_{fo}")
                o = fo * KD * P
                if fo == 2:
                    pace(nc.sync, 3)
                elif fo == 3:
                    pace(nc.sync, 1)
                elif fo == 5:
                    pace(nc.sync, 1)
                nc.sync.dma_start(wtile[:], w0t[:, o:o + KD * P])
                w0_sb.append(wtile)
            w1_all = w1pool.tile([P, 4 * KH * 512], F16, tag="w1", name="w1")
            nc.sync.dma_start(w1_all[:], w1t[:, :])

            # PE p-state warmup: the tensor engine idles at ~1.2GHz and
            # ramps to 2.4GHz over ~3us of continuous activity. Junk
            # matmuls during the x0/w0_0 DMA wait start the ramp early so
            # the real ramp runs at full clock.
            wps = psum.tile([P, P], F32, tag="ps", name="warm_ps")
            for r in range(48):
                nc.tensor.matmul(wps[:, :64], warm[:], warm[:, :64],
                                 start=(r == 0), stop=(r == 47))
            wjunk = const.tile([P, 32], F32, name="warm_out")
            nc.vector.tensor_copy(wjunk[:], wps[:, :32])

            # hT = gelu(x @ w0 + b0), laid out [F-part, T-free], fp16
            h_sb = hpool.tile([P, KF, TO * P], F16)

            def x_k(t, k):
                return x_sb[t][:, k * t_chunks[t]:(k + 1) * t_chunks[t]]

            def w0_k(fo, k):
                return w0_sb[fo][:, k * P:(k + 1) * P]

            def act(fo, t, ps):
                nc.scalar.activation(h_sb[:, fo, t_offs[t]:t_offs[t + 1]], ps,
                                     mybir.ActivationFunctionType.Gelu,
                                     bias=b0_sb[:, fo:fo + 1])

            # ---- phase 1: mm1 + gelu ----
            # ramp: ordered to match measured DMA arrivals — x0, w0_0..2
            # land first, then x1, then w0_3/w0_4 (delayed while x1/x2 hog
            # rings), then x2. Each group is k-inner self-loading.
            if NTC >= 3:
                ramp_pairs = [(f, 0) for f in range(3)]
                ramp_pairs += [(f, 1) for f in range(3)]
                ramp_pairs += [(3, 0), (3, 1), (4, 0), (4, 1)]
                ramp_pairs += [(f, t) for t in range(2, NTC)
                               for f in range(RAMP_FOS)]
            else:
                ramp_pairs = [(f, t) for t in range(NTC)
                              for f in range(RAMP_FOS)]
            for fo, t in ramp_pairs:
                ps = psum.tile([P, 512], F32, tag="ps",
                               name=f"ps1_{fo}_{t}")[:, :t_chunks[t]]
                for k in range(KD):
                    nc.tensor.matmul(ps, w0_k(fo, k), x_k(t, k),
                                     start=(k == 0), stop=(k == KD - 1))
                act(fo, t, ps)

            # steady: k-outer/t-inner — each w0 k-tile streams all chunks
            for fo in range(RAMP_FOS, KF):
                if mode == "kin":
                    for t in range(NTC):
                        ps = psum.tile([P, 512], F32, tag="ps",
                                       name=f"ps1_{fo}_{t}")[:, :t_chunks[t]]
                        for k in range(KD):
                            nc.tensor.matmul(ps, w0_k(fo, k), x_k(t, k),
                                             start=(k == 0),
                                             stop=(k == KD - 1))
                        act(fo, t, ps)
                else:
                    pss = [psum.tile([P, 512], F32, tag="ps",
                                     name=f"ps1_{fo}_{t}")[:, :t_chunks[t]]
                           for t in range(NTC)]
                    for k in range(KD):
                        for t in range(NTC):
                            nc.tensor.matmul(pss[t], w0_k(fo, k), x_k(t, k),
                                             start=(k == 0),
                                             stop=(k == KD - 1))
                    for t in range(NTC):
                        act(fo, t, pss[t])

            # ---- phase 2: mm2 ----
            # the very last output group is split into two 256-wide halves
            # so the end-of-kernel copy+DMA tail is half as long
            for dc, DCH in enumerate(D_CHUNKS):
                for to in range(TO):
                    last = (dc == len(D_CHUNKS) - 1) and (to == TO - 1)
                    for half in ([None] if not last else [0, 1]):
                        if half is None:
                            d0, dw = 0, 512
                        else:
                            d0, dw = half * 256, 256
                        ps2 = psum.tile([P, 512], F32, tag="ps",
                                        name=f"ps2_{dc}_{to}_{d0}")[:, :dw]
                        for k in range(KF):
                            o = ((dc * 2 + k // KH) * KH
                                 + k % KH) * 512 + d0
                            nc.tensor.matmul(
                                ps2, h_sb[:, k, to * P:(to + 1) * P],
                                w1_all[:, o:o + dw],
                                start=(k == 0), stop=(k == KF - 1))
                        y_sb = ypool.tile([P, 512], F16, tag="y",
                                          name=f"y_{dc}_{to}_{d0}")[:, :dw]
                        nc.vector.tensor_copy(y_sb, ps2)
                        nc.sync.dma_start(
                            y[to * P:(to + 1) * P,
                              dc * 512 + d0:dc * 512 + d0 + dw], y_sb)

    nc.compile()
    return nc


def _chunks_for(T):
    """mm1 moving-dim chunks: small leading chunks for the DMA/PE ramp,
    512 (the PSUM bank limit) thereafter."""
    t_chunks = []
    rest = T
    for c in (256, 384):
        if rest <= 0:
            break
        t_chunks.append(min(c, rest))
        rest -= t_chunks[-1]
    while rest > 0:
        c = min(512, rest)
        t_chunks.append(c)
        rest -= c
    return t_chunks


def _prep_inputs(x, w0, b0, w1, idx, cnt, T, t_chunks, t_offs):
    """Host-side gather + fp16 cast + retile into per-partition-contiguous
    DMA blocks for one expert."""
    # x chunks: [P, KD*tc] blocks, chunk-major
    xT = np.zeros((KD, P, T), dtype=np.float16)
    xT.reshape(D, T)[:, :cnt] = x[idx].T
    parts = [xT[:, :, t_offs[c]:t_offs[c + 1]].transpose(1, 0, 2)
             .reshape(P, -1) for c in range(len(t_chunks))]
    xt = np.ascontiguousarray(np.concatenate(parts, axis=1))
    # w0 tiles: per fo [P, KD*128]
    w0t = np.ascontiguousarray(
        w0.astype(np.float16).reshape(KD, P, KF, P)
        .transpose(1, 2, 0, 3).reshape(P, KF * KD * P))
    # w1 tiles: per (dc, kh) [P, KH*512]
    w1t = np.ascontiguousarray(
        w1.astype(np.float16).reshape(2, KH, P, 2, 512)
        .transpose(2, 3, 0, 1, 4).reshape(P, 4 * KH * 512))
    b0t = np.ascontiguousarray(b0[0].astype(np.float32).reshape(KF, P).T)
    return {"xt": xt, "w0t": w0t, "w1t": w1t, "b0": b0t}


def kernel(x, routing_tensor, w0, b0, w1, b1):
    x = np.ascontiguousarray(np.asarray(x, dtype=np.float32))
    routing = np.asarray(routing_tensor, dtype=np.float32)
    w0 = np.asarray(w0, dtype=np.float32)
    b0 = np.asarray(b0, dtype=np.float32)
    w1 = np.asarray(w1, dtype=np.float32)
    b1 = np.asarray(b1, dtype=np.float32)

    idx = [np.nonzero(routing[:, e])[0] for e in range(E)]
    cnt = [len(i) for i in idx]
    T = max(256, -(-max(cnt) // 16) * 16)   # mm1 columns (pad to 16)
    t_chunks = _chunks_for(T)
    t_offs = [0, *np.cumsum(t_chunks).tolist()]

    nc = _cache.get((T, MM1_MODE))
    if nc is None:
        nc = _cache[T, MM1_MODE] = build_program(T, MM1_MODE)

    in_maps = [_prep_inputs(x, w0[e], b0[e], w1[e], idx[e], cnt[e], T,
                            t_chunks, t_offs) for e in range(E)]

    res = run_bass_kernel_spmd(nc, in_maps, core_ids=list(range(E)))

    # combine: out = sum_e r_e * (y_e + b1_e)
    out = routing @ b1[:, 0, :]
    for e in range(E):
        r = routing[idx[e], e:e + 1]
        out[idx[e]] += r * res.results[e]["y"][:cnt[e]].astype(np.float32)
    return out.astype(np.float32)
